# revision 1
# baseline (speedup 1.0000x reference)
"""MeshPooling Trainium2 kernel (nn_MeshPooling_34479997452437).

Full-input contract: kernel(**inputs) takes the complete tensors and returns
the full reference-shaped output tuple. Internally the nodes are sharded
across 8 NeuronCores by cluster ownership (cluster c -> core c // 15625).

Device computes the segment-sum (scatter-mean numerator + counts):
- Host snake-deals each core's 15625 clusters into 128 windows of 128
  cluster-slots, balancing node counts per window (~4.9% padding).
- Node features are packed fp16: x(128) xpos_hi(2) idx 3x5 base-256 digits
  (exact in fp16) batch(1) ones(1) | xpos_lo(2). Integer columns are exact:
  digit sums accumulate exactly in fp32 PSUM and are recombined on the host,
  so int64 outputs are bit-exact vs the fp32 reference.
- Per window: one DVE is_equal builds the 0/1 selection matrix S.T for all
  node tiles; per 128-node tile two fp16 matmuls accumulate into PSUM
  [128 clusters, 147] (xpos_lo folded into xpos_hi columns by PSUM).
- Raw sums stream out; the host performs the scatter-mean division in fp32
  (bit-identical to the reference's sums / max(counts, 1)).

Edge coalesce (cluster[edge_index] -> unique sorted keys, padded) runs on the
host: the int64 key space (1.56e10) exceeds what device engines sort
efficiently, and it is a small fraction of total memory traffic.
"""
import sys, os, time

for _p in ('/opt/trn_rl_repo', '/root/.axon_site/_ro/trn_rl_repo'):
    if os.path.isdir(_p) and _p not in sys.path:
        sys.path.insert(0, _p)

import numpy as np

N = 500_000
E = 3_000_000
C = 128
M = 125_000
NUMPOOL = 5
NCORES = 8
MPC = M // NCORES          # 15625 clusters per core
WIN = 128
NWIN = 128                 # windows per core (balanced via snake-deal)
G = 4                      # windows per input DMA batch
NB = NWIN // G             # 32 DMA batches per core
NWP = NWIN
MP = NWP * WIN             # padded clusters per core (16384)
FA = C + 2 + 3 * NUMPOOL + 1 + 1   # 147 A-section cols
FB = 2                              # B-section: xpos_lo only
F2 = FA + FB                        # 149 input cols per node
FO = FA                             # 147 output cols


def _split_waits(nc, mybir, maxw=1):
    """This container's walrus rejects >1 sync wait per instruction; split
    extras into preceding wait-only EventSemaphore ops on the same engine."""
    for fn in nc.m.functions:
        for bb in fn.blocks:
            new_insts = []
            for inst in bb.instructions:
                si = getattr(inst, 'sync_info', None)
                if si is not None and si.on_wait and len(si.on_wait) > maxw:
                    waits = list(si.on_wait)
                    extra, keep = waits[:-maxw], waits[-maxw:]
                    for w in extra:
                        ev = mybir.InstEventSemaphore(
                            name=nc.get_next_instruction_name(),
                            engine=inst.engine, ins=[], outs=[])
                        ev.sync_info = mybir.SyncInfo(on_wait=[w], on_update=[])
                        new_insts.append(ev)
                    si.on_wait.clear()
                    for w in keep:
                        si.on_wait.append(w)
                new_insts.append(inst)
            bb.instructions[:] = new_insts


def _build_nc(T):
    import concourse.bass as bass
    import concourse.mybir as mybir
    import concourse.tile as tile
    from contextlib import ExitStack

    nc = bass.Bass()
    # batch-contiguous: row (b*128+p) holds G windows x T tiles x F2 fp16
    feats = nc.dram_tensor("feats", [NB * 128, G * T * F2], mybir.dt.float16,
                           kind="ExternalInput")
    crel = nc.dram_tensor("crel", [128, NWP * T], mybir.dt.float32,
                          kind="ExternalInput")
    iota = nc.dram_tensor("iota", [128, 128], mybir.dt.float32,
                          kind="ExternalInput")
    # row (b*128+p) holds the G windows' FO-wide raw-sum rows for slot p
    out = nc.dram_tensor("out", [NB * 128, G * FO], mybir.dt.float32,
                         kind="ExternalOutput")

    with tile.TileContext(nc) as tc:
        with ExitStack() as ctx:
            const_pool = ctx.enter_context(tc.tile_pool(name="const", bufs=1))
            in_pool = ctx.enter_context(tc.tile_pool(name="in", bufs=4))
            st_pool = ctx.enter_context(tc.tile_pool(name="st", bufs=6))
            out_pool = ctx.enter_context(tc.tile_pool(name="outp", bufs=3))
            psum_pool = ctx.enter_context(
                tc.tile_pool(name="psum", bufs=6, space="PSUM"))

            iota_t = const_pool.tile([128, 128], mybir.dt.float32)
            nc.sync.dma_start(iota_t[:], iota[:])
            crel_t = const_pool.tile([128, NWP * T], mybir.dt.float32)
            nc.sync.dma_start(crel_t[:], crel[:])

            for b in range(NB):
                ft = in_pool.tile([128, G * T * F2], mybir.dt.float16, tag="ft")
                nc.sync.dma_start(ft[:], feats[b * 128:(b + 1) * 128, :])
                og = out_pool.tile([128, G * FO], mybir.dt.float32, tag="og")
                for wi in range(G):
                    w = b * G + wi
                    fw = ft[:, wi * T * F2:(wi + 1) * T * F2]

                    # S.T for the window's T tiles in one DVE op:
                    # st[p, t, q] = (crel[p, w*T+t] == q)
                    st = st_pool.tile([128, T * 128], mybir.dt.float16,
                                      tag="st")
                    st3 = st[:].rearrange("p (t q) -> p t q", q=128)
                    crel_b = crel_t[:, w * T:(w + 1) * T].to_broadcast(
                        [128, T, 128])
                    iota_b = bass.AP(iota_t[:].tensor, iota_t[:].offset,
                                     [iota_t[:].ap[0], [0, T],
                                      iota_t[:].ap[1]])
                    nc.vector.tensor_tensor(out=st3, in0=crel_b, in1=iota_b,
                                            op=mybir.AluOpType.is_equal)

                    ps = psum_pool.tile([128, FA], mybir.dt.float32)
                    for t in range(T):
                        lhsT = st[:, t * 128:(t + 1) * 128]
                        nc.tensor.matmul(
                            out=ps[:, 0:FA], lhsT=lhsT,
                            rhs=fw[:, t * F2:t * F2 + FA],
                            start=(t == 0), stop=False)
                        # xpos_lo folds into the xpos_hi psum columns
                        nc.tensor.matmul(
                            out=ps[:, C:C + 2], lhsT=lhsT,
                            rhs=fw[:, t * F2 + FA:(t + 1) * F2],
                            start=False, stop=(t == T - 1))

                    # raw sums out; all division happens on the host
                    nc.scalar.copy(og[:, wi * FO:(wi + 1) * FO], ps[:, 0:FA])
                nc.scalar.dma_start(out[b * 128:(b + 1) * 128, :], og[:])

    import concourse.mybir as mybir2
    _split_waits(nc, mybir2)
    return nc


def _host_prepare(x, xpos, indexattten, batch, poolindex):
    poolindex = int(poolindex)
    cluster = np.asarray(indexattten)[:, poolindex].astype(np.int32)
    cl64 = cluster.astype(np.int64)
    core_of = cl64 // MPC
    c_local = cl64 - core_of * MPC

    # balance node counts across NWIN windows per core: snake-deal clusters
    # (sorted by count desc) into windows; slot-in-window = deal round.
    cnt_cc = np.zeros((NCORES, MPC), np.int32)
    np.add.at(cnt_cc, (core_of, c_local), 1)
    order_c = np.argsort(-cnt_cc, axis=1, kind='stable')
    ridx = np.arange(MPC)
    rnd = ridx // NWIN
    pos = ridx % NWIN
    w_sorted = np.where(rnd % 2 == 0, pos, NWIN - 1 - pos)
    w_c = np.empty((NCORES, MPC), np.int32)
    j_c = np.empty((NCORES, MPC), np.int32)
    rows = np.arange(NCORES)[:, None]
    w_c[rows, order_c] = w_sorted[None, :]
    j_c[rows, order_c] = rnd[None, :]

    w_local = w_c[core_of, c_local].astype(np.int64)
    c_rel = j_c[core_of, c_local].astype(np.float32)
    gw = (core_of * NWIN + w_local).astype(np.int64)

    counts = np.bincount(gw, minlength=NCORES * NWIN)
    Kmax = int(counts.max())
    K = max(128, ((Kmax + 127) // 128) * 128)
    T = K // 128
    S = NWP * K

    order = np.argsort(gw, kind='stable')
    gws = gw[order]
    starts = np.zeros(NCORES * NWIN + 1, dtype=np.int64)
    np.cumsum(counts, out=starts[1:])
    rank = np.arange(N, dtype=np.int64) - starts[gws]
    p_of = rank % 128
    t_of = rank // 128
    wl = w_local[order]
    b_of = wl // G
    wi_of = wl - b_of * G
    slot_in_core = b_of * (128 * G * T) + p_of * (G * T) + wi_of * T + t_of
    global_slot = core_of[order] * S + slot_in_core

    F16 = np.float16
    xo = np.asarray(x)[order].astype(F16)
    xpo = np.asarray(xpos)[order]
    xp_hi = xpo.astype(F16)
    xp_lo = (xpo - xp_hi.astype(np.float32)).astype(F16)
    vi = np.asarray(indexattten)[order].astype(np.int32)
    idx_parts = np.empty((N, 3 * NUMPOOL), F16)
    idx_parts[:, 0::3] = (vi >> 16).astype(F16)
    idx_parts[:, 1::3] = ((vi >> 8) & 255).astype(F16)
    idx_parts[:, 2::3] = (vi & 255).astype(F16)

    feats_pad = np.zeros((NCORES * S, F2), F16)
    feats_pad[global_slot, 0:C] = xo
    feats_pad[global_slot, C:C + 2] = xp_hi
    feats_pad[global_slot, C + 2:C + 2 + 3 * NUMPOOL] = idx_parts
    feats_pad[global_slot, FA - 2] = np.asarray(batch)[order].astype(F16)
    feats_pad[global_slot, FA - 1] = F16(1.0)
    feats_pad[global_slot, FA:FA + 2] = xp_lo
    feats_pad = feats_pad.reshape(NCORES, NB * 128, G * T * F2)

    crel_slot = np.full(NCORES * S, -1.0, np.float32)
    crel_slot[global_slot] = c_rel[order]
    crel_T = np.ascontiguousarray(
        crel_slot.reshape(NCORES, NB, 128, G, T).transpose(0, 2, 1, 3, 4)
        .reshape(NCORES, 128, NWP * T))

    iota = np.tile(np.arange(128, dtype=np.float32), (128, 1))
    # device flat row (out viewed as [(NB*128*G), FO]) of local cluster c:
    # row = (b*128 + j)*G + wi with b = w//G, wi = w%G
    w64 = w_c.astype(np.int64)
    out_row = ((w64 // G) * 128 + j_c) * G + (w64 % G)
    return cluster, feats_pad, crel_T, iota, T, out_row


_nc_cache = {}


def _run_device(feats_pad, crel_T, iota, T, trace=False, tmpdir=None):
    from concourse.bass_utils import run_bass_kernel_spmd
    if T not in _nc_cache:
        _nc_cache[T] = _build_nc(T)
    nc = _nc_cache[T]
    in_maps = [
        {"feats": feats_pad[i], "crel": crel_T[i], "iota": iota}
        for i in range(NCORES)
    ]
    return run_bass_kernel_spmd(nc, in_maps, core_ids=list(range(NCORES)),
                                trace=trace, tmpdir=tmpdir)


def _segment_sums_numpy(x, xpos, indexattten, batch, cluster):
    """Host fallback producing the same [M, FO] raw-sum layout."""
    big = np.zeros((M, FO), np.float32)
    feats = np.concatenate([
        np.asarray(x, np.float32),
        np.asarray(xpos, np.float32),
        np.zeros((N, 3 * NUMPOOL), np.float32),
        np.asarray(batch, np.float32)[:, None],
        np.ones((N, 1), np.float32)], axis=1)
    vi = np.asarray(indexattten).astype(np.int32)
    feats[:, C + 2:C + 2 + 3 * NUMPOOL:3] = (vi >> 16).astype(np.float32)
    feats[:, C + 3:C + 2 + 3 * NUMPOOL:3] = ((vi >> 8) & 255).astype(np.float32)
    feats[:, C + 4:C + 2 + 3 * NUMPOOL:3] = (vi & 255).astype(np.float32)
    np.add.at(big, cluster, feats)
    return big


def kernel(x, edge_index, xpos, indexattten, batch, poolindex,
           trace=False, tmpdir=None, _times=None):
    x = np.asarray(x)
    edge_index = np.asarray(edge_index)
    xpos = np.asarray(xpos)
    indexattten = np.asarray(indexattten)
    batch = np.asarray(batch)

    t0 = time.time()
    cluster, feats_pad, crel_T, iota, T, out_row = _host_prepare(
        x, xpos, indexattten, batch, poolindex)
    t1 = time.time()

    big = None
    res = None
    try:
        res = _run_device(feats_pad, crel_T, iota, T, trace=trace,
                          tmpdir=tmpdir)
        big = np.concatenate(
            [res.results[i]["out"].reshape(NB * 128 * G, FO)[out_row[i]]
             for i in range(NCORES)], axis=0)
    except Exception as e:
        sys.stderr.write(f"[kernel] device path failed ({e!r}); "
                         f"falling back to host compute\n")
        big = _segment_sums_numpy(x, xpos, indexattten, batch, cluster)
    t2 = time.time()

    counts = big[:, FO - 1:FO]
    denom = np.maximum(counts, np.float32(1.0))
    ip = big[:, C + 2:C + 2 + 3 * NUMPOOL]
    idx_sums = (ip[:, 0::3] * np.float32(65536.0)
                + ip[:, 1::3] * np.float32(256.0) + ip[:, 2::3])
    indexatttennew = (idx_sums / denom).astype(np.int64)
    new_batch = (big[:, FO - 2:FO - 1] / denom)[:, 0].astype(np.int64)
    new_xfinal = big[:, :C] / denom
    new_pos = big[:, C:C + 2] / denom
    t3 = time.time()

    cl64 = cluster.astype(np.int64)
    keys = cl64[edge_index[0]] * M + cl64[edge_index[1]]
    uk = np.unique(keys)
    full = np.empty(E, dtype=np.int64)
    full[:len(uk)] = uk
    full[len(uk):] = uk[-1]
    new_edge_index = np.stack([full // M, full % M])
    t4 = time.time()
    if _times is not None:
        _times.update(dict(prep=t1 - t0, device=t2 - t1, post=t3 - t2,
                           edges=t4 - t3))
    out = (new_xfinal, new_edge_index, new_pos, indexatttennew, new_batch,
           edge_index, cluster, batch)
    if trace:
        return out, res
    return out


# revision 2
# speedup vs baseline: 1.0671x; 1.0671x over previous
"""MeshPooling Trainium2 kernel (nn_MeshPooling_34479997452437).

Full-input contract: kernel(**inputs) takes the complete tensors and returns
the full reference-shaped output tuple. Internally the nodes are sharded
across 8 NeuronCores by cluster ownership (cluster c -> core c // 15625).

Device computes the segment-sum (scatter-mean numerator + counts):
- Host snake-deals each core's 15625 clusters into 128 windows of 128
  cluster-slots, balancing node counts per window (~4.9% padding).
- Node features are packed fp16: x(128) xpos_hi(2) idx 3x5 base-256 digits
  (exact in fp16) batch(1) ones(1) | xpos_lo(2). Integer columns are exact:
  digit sums accumulate exactly in fp32 PSUM and are recombined on the host,
  so int64 outputs are bit-exact vs the fp32 reference.
- Per window: one DVE is_equal builds the 0/1 selection matrix S.T for all
  node tiles; per 128-node tile two fp16 matmuls accumulate into PSUM
  [128 clusters, 147] (xpos_lo folded into xpos_hi columns by PSUM).
- Raw sums stream out; the host performs the scatter-mean division in fp32
  (bit-identical to the reference's sums / max(counts, 1)).

Edge coalesce (cluster[edge_index] -> unique sorted keys, padded) runs on the
host: the int64 key space (1.56e10) exceeds what device engines sort
efficiently, and it is a small fraction of total memory traffic.
"""
import sys, os, time

for _p in ('/opt/trn_rl_repo', '/root/.axon_site/_ro/trn_rl_repo'):
    if os.path.isdir(_p) and _p not in sys.path:
        sys.path.insert(0, _p)

import numpy as np

N = 500_000
E = 3_000_000
C = 128
M = 125_000
NUMPOOL = 5
NCORES = 8
MPC = M // NCORES          # 15625 clusters per core
WIN = 128
NWIN = 128                 # windows per core (balanced via snake-deal)
G = 4                      # windows per input DMA batch
NB = NWIN // G             # 32 DMA batches per core
NWP = NWIN
MP = NWP * WIN             # padded clusters per core (16384)
FA = C + 2 + 3 * NUMPOOL + 1 + 1   # 147 A-section cols
FB = 2                              # B-section: xpos_lo only
F2 = FA + FB                        # 149 input cols per node
FO = FA                             # 147 output cols


def _split_waits(nc, mybir, maxw=1):
    """This container's walrus rejects >1 sync wait per instruction; split
    extras into preceding wait-only EventSemaphore ops on the same engine."""
    for fn in nc.m.functions:
        for bb in fn.blocks:
            new_insts = []
            for inst in bb.instructions:
                si = getattr(inst, 'sync_info', None)
                if si is not None and si.on_wait and len(si.on_wait) > maxw:
                    waits = list(si.on_wait)
                    extra, keep = waits[:-maxw], waits[-maxw:]
                    for w in extra:
                        ev = mybir.InstEventSemaphore(
                            name=nc.get_next_instruction_name(),
                            engine=inst.engine, ins=[], outs=[])
                        ev.sync_info = mybir.SyncInfo(on_wait=[w], on_update=[])
                        new_insts.append(ev)
                    si.on_wait.clear()
                    for w in keep:
                        si.on_wait.append(w)
                new_insts.append(inst)
            bb.instructions[:] = new_insts


def _build_nc(T):
    import concourse.bass as bass
    import concourse.mybir as mybir
    import concourse.tile as tile
    from contextlib import ExitStack

    nc = bass.Bass()
    # batch-contiguous: row (b*128+p) holds G windows x T tiles x F2 fp16
    feats = nc.dram_tensor("feats", [NB * 128, G * T * F2], mybir.dt.float16,
                           kind="ExternalInput")
    crel = nc.dram_tensor("crel", [128, NWP * T], mybir.dt.float32,
                          kind="ExternalInput")
    iota = nc.dram_tensor("iota", [128, 128], mybir.dt.float32,
                          kind="ExternalInput")
    # row (b*128+p) holds the G windows' FO-wide raw-sum rows for slot p
    out = nc.dram_tensor("out", [NB * 128, G * FO], mybir.dt.float32,
                         kind="ExternalOutput")

    with tile.TileContext(nc) as tc:
        with ExitStack() as ctx:
            const_pool = ctx.enter_context(tc.tile_pool(name="const", bufs=1))
            in_pool = ctx.enter_context(tc.tile_pool(name="in", bufs=6))
            st_pool = ctx.enter_context(tc.tile_pool(name="st", bufs=8))
            out_pool = ctx.enter_context(tc.tile_pool(name="outp", bufs=4))
            psum_pool = ctx.enter_context(
                tc.tile_pool(name="psum", bufs=8, space="PSUM"))

            iota_t = const_pool.tile([128, 128], mybir.dt.float32)
            nc.sync.dma_start(iota_t[:], iota[:])
            crel_t = const_pool.tile([128, NWP * T], mybir.dt.float32)
            nc.sync.dma_start(crel_t[:], crel[:])

            for b in range(NB):
                ft = in_pool.tile([128, G * T * F2], mybir.dt.float16, tag="ft")
                nc.sync.dma_start(ft[:], feats[b * 128:(b + 1) * 128, :])
                og = out_pool.tile([128, G * FO], mybir.dt.float32, tag="og")
                for wi in range(G):
                    w = b * G + wi
                    fw = ft[:, wi * T * F2:(wi + 1) * T * F2]

                    # S.T for the window's T tiles in one DVE op:
                    # st[p, t, q] = (crel[p, w*T+t] == q)
                    st = st_pool.tile([128, T * 128], mybir.dt.float16,
                                      tag="st")
                    st3 = st[:].rearrange("p (t q) -> p t q", q=128)
                    crel_b = crel_t[:, w * T:(w + 1) * T].to_broadcast(
                        [128, T, 128])
                    iota_b = bass.AP(iota_t[:].tensor, iota_t[:].offset,
                                     [iota_t[:].ap[0], [0, T],
                                      iota_t[:].ap[1]])
                    nc.vector.tensor_tensor(out=st3, in0=crel_b, in1=iota_b,
                                            op=mybir.AluOpType.is_equal)

                    ps = psum_pool.tile([128, FA], mybir.dt.float32)
                    for t in range(T):
                        lhsT = st[:, t * 128:(t + 1) * 128]
                        nc.tensor.matmul(
                            out=ps[:, 0:FA], lhsT=lhsT,
                            rhs=fw[:, t * F2:t * F2 + FA],
                            start=(t == 0), stop=False)
                        # xpos_lo folds into the xpos_hi psum columns
                        nc.tensor.matmul(
                            out=ps[:, C:C + 2], lhsT=lhsT,
                            rhs=fw[:, t * F2 + FA:(t + 1) * F2],
                            start=False, stop=(t == T - 1))

                    # raw sums out; all division happens on the host
                    nc.scalar.copy(og[:, wi * FO:(wi + 1) * FO], ps[:, 0:FA])
                nc.scalar.dma_start(out[b * 128:(b + 1) * 128, :], og[:])

    import concourse.mybir as mybir2
    _split_waits(nc, mybir2)
    return nc


def _host_prepare(x, xpos, indexattten, batch, poolindex):
    poolindex = int(poolindex)
    cluster = np.asarray(indexattten)[:, poolindex].astype(np.int32)
    cl64 = cluster.astype(np.int64)
    core_of = cl64 // MPC
    c_local = cl64 - core_of * MPC

    # balance node counts across NWIN windows per core: snake-deal clusters
    # (sorted by count desc) into windows; slot-in-window = deal round.
    cnt_cc = np.zeros((NCORES, MPC), np.int32)
    np.add.at(cnt_cc, (core_of, c_local), 1)
    order_c = np.argsort(-cnt_cc, axis=1, kind='stable')
    ridx = np.arange(MPC)
    rnd = ridx // NWIN
    pos = ridx % NWIN
    w_sorted = np.where(rnd % 2 == 0, pos, NWIN - 1 - pos)
    w_c = np.empty((NCORES, MPC), np.int32)
    j_c = np.empty((NCORES, MPC), np.int32)
    rows = np.arange(NCORES)[:, None]
    w_c[rows, order_c] = w_sorted[None, :]
    j_c[rows, order_c] = rnd[None, :]

    w_local = w_c[core_of, c_local].astype(np.int64)
    c_rel = j_c[core_of, c_local].astype(np.float32)
    gw = (core_of * NWIN + w_local).astype(np.int64)

    counts = np.bincount(gw, minlength=NCORES * NWIN)
    Kmax = int(counts.max())
    K = max(128, ((Kmax + 127) // 128) * 128)
    T = K // 128
    S = NWP * K

    order = np.argsort(gw, kind='stable')
    gws = gw[order]
    starts = np.zeros(NCORES * NWIN + 1, dtype=np.int64)
    np.cumsum(counts, out=starts[1:])
    rank = np.arange(N, dtype=np.int64) - starts[gws]
    p_of = rank % 128
    t_of = rank // 128
    wl = w_local[order]
    b_of = wl // G
    wi_of = wl - b_of * G
    slot_in_core = b_of * (128 * G * T) + p_of * (G * T) + wi_of * T + t_of
    global_slot = core_of[order] * S + slot_in_core

    F16 = np.float16
    xo = np.asarray(x)[order].astype(F16)
    xpo = np.asarray(xpos)[order]
    xp_hi = xpo.astype(F16)
    xp_lo = (xpo - xp_hi.astype(np.float32)).astype(F16)
    vi = np.asarray(indexattten)[order].astype(np.int32)
    idx_parts = np.empty((N, 3 * NUMPOOL), F16)
    idx_parts[:, 0::3] = (vi >> 16).astype(F16)
    idx_parts[:, 1::3] = ((vi >> 8) & 255).astype(F16)
    idx_parts[:, 2::3] = (vi & 255).astype(F16)

    feats_pad = np.zeros((NCORES * S, F2), F16)
    feats_pad[global_slot, 0:C] = xo
    feats_pad[global_slot, C:C + 2] = xp_hi
    feats_pad[global_slot, C + 2:C + 2 + 3 * NUMPOOL] = idx_parts
    feats_pad[global_slot, FA - 2] = np.asarray(batch)[order].astype(F16)
    feats_pad[global_slot, FA - 1] = F16(1.0)
    feats_pad[global_slot, FA:FA + 2] = xp_lo
    feats_pad = feats_pad.reshape(NCORES, NB * 128, G * T * F2)

    crel_slot = np.full(NCORES * S, -1.0, np.float32)
    crel_slot[global_slot] = c_rel[order]
    crel_T = np.ascontiguousarray(
        crel_slot.reshape(NCORES, NB, 128, G, T).transpose(0, 2, 1, 3, 4)
        .reshape(NCORES, 128, NWP * T))

    iota = np.tile(np.arange(128, dtype=np.float32), (128, 1))
    # device flat row (out viewed as [(NB*128*G), FO]) of local cluster c:
    # row = (b*128 + j)*G + wi with b = w//G, wi = w%G
    w64 = w_c.astype(np.int64)
    out_row = ((w64 // G) * 128 + j_c) * G + (w64 % G)
    return cluster, feats_pad, crel_T, iota, T, out_row


_nc_cache = {}


def _run_device(feats_pad, crel_T, iota, T, trace=False, tmpdir=None):
    from concourse.bass_utils import run_bass_kernel_spmd
    if T not in _nc_cache:
        _nc_cache[T] = _build_nc(T)
    nc = _nc_cache[T]
    in_maps = [
        {"feats": feats_pad[i], "crel": crel_T[i], "iota": iota}
        for i in range(NCORES)
    ]
    return run_bass_kernel_spmd(nc, in_maps, core_ids=list(range(NCORES)),
                                trace=trace, tmpdir=tmpdir)


def _segment_sums_numpy(x, xpos, indexattten, batch, cluster):
    """Host fallback producing the same [M, FO] raw-sum layout."""
    big = np.zeros((M, FO), np.float32)
    feats = np.concatenate([
        np.asarray(x, np.float32),
        np.asarray(xpos, np.float32),
        np.zeros((N, 3 * NUMPOOL), np.float32),
        np.asarray(batch, np.float32)[:, None],
        np.ones((N, 1), np.float32)], axis=1)
    vi = np.asarray(indexattten).astype(np.int32)
    feats[:, C + 2:C + 2 + 3 * NUMPOOL:3] = (vi >> 16).astype(np.float32)
    feats[:, C + 3:C + 2 + 3 * NUMPOOL:3] = ((vi >> 8) & 255).astype(np.float32)
    feats[:, C + 4:C + 2 + 3 * NUMPOOL:3] = (vi & 255).astype(np.float32)
    np.add.at(big, cluster, feats)
    return big


def kernel(x, edge_index, xpos, indexattten, batch, poolindex,
           trace=False, tmpdir=None, _times=None):
    x = np.asarray(x)
    edge_index = np.asarray(edge_index)
    xpos = np.asarray(xpos)
    indexattten = np.asarray(indexattten)
    batch = np.asarray(batch)

    t0 = time.time()
    cluster, feats_pad, crel_T, iota, T, out_row = _host_prepare(
        x, xpos, indexattten, batch, poolindex)
    t1 = time.time()

    big = None
    res = None
    try:
        res = _run_device(feats_pad, crel_T, iota, T, trace=trace,
                          tmpdir=tmpdir)
        big = np.concatenate(
            [res.results[i]["out"].reshape(NB * 128 * G, FO)[out_row[i]]
             for i in range(NCORES)], axis=0)
    except Exception as e:
        sys.stderr.write(f"[kernel] device path failed ({e!r}); "
                         f"falling back to host compute\n")
        big = _segment_sums_numpy(x, xpos, indexattten, batch, cluster)
    t2 = time.time()

    counts = big[:, FO - 1:FO]
    denom = np.maximum(counts, np.float32(1.0))
    ip = big[:, C + 2:C + 2 + 3 * NUMPOOL]
    idx_sums = (ip[:, 0::3] * np.float32(65536.0)
                + ip[:, 1::3] * np.float32(256.0) + ip[:, 2::3])
    indexatttennew = (idx_sums / denom).astype(np.int64)
    new_batch = (big[:, FO - 2:FO - 1] / denom)[:, 0].astype(np.int64)
    new_xfinal = big[:, :C] / denom
    new_pos = big[:, C:C + 2] / denom
    t3 = time.time()

    cl64 = cluster.astype(np.int64)
    keys = cl64[edge_index[0]] * M + cl64[edge_index[1]]
    uk = np.unique(keys)
    full = np.empty(E, dtype=np.int64)
    full[:len(uk)] = uk
    full[len(uk):] = uk[-1]
    new_edge_index = np.stack([full // M, full % M])
    t4 = time.time()
    if _times is not None:
        _times.update(dict(prep=t1 - t0, device=t2 - t1, post=t3 - t2,
                           edges=t4 - t3))
    out = (new_xfinal, new_edge_index, new_pos, indexatttennew, new_batch,
           edge_index, cluster, batch)
    if trace:
        return out, res
    return out


# revision 3
# speedup vs baseline: 1.0966x; 1.0276x over previous
"""MeshPooling Trainium2 kernel (nn_MeshPooling_34479997452437).

Full-input contract: kernel(**inputs) takes the complete tensors and returns
the full reference-shaped output tuple. Internally the nodes are sharded
across 8 NeuronCores by cluster ownership (cluster c -> core c // 15625).

Device computes the segment-sum (scatter-mean numerator + counts):
- Host snake-deals each core's 15625 clusters into 128 windows of 128
  cluster-slots, balancing node counts per window (~4.9% padding).
- Node features are packed fp16: x(128) xpos_hi(2) idx 3x5 base-256 digits
  (exact in fp16) batch(1) ones(1) | xpos_lo(2). Integer columns are exact:
  digit sums accumulate exactly in fp32 PSUM and are recombined on the host,
  so int64 outputs are bit-exact vs the fp32 reference.
- Per window: one DVE is_equal builds the 0/1 selection matrix S.T for all
  node tiles; per 128-node tile two fp16 matmuls accumulate into PSUM
  [128 clusters, 147] (xpos_lo folded into xpos_hi columns by PSUM).
- Raw sums stream out; the host performs the scatter-mean division in fp32
  (bit-identical to the reference's sums / max(counts, 1)).

Edge coalesce (cluster[edge_index] -> unique sorted keys, padded) runs on the
host: the int64 key space (1.56e10) exceeds what device engines sort
efficiently, and it is a small fraction of total memory traffic.
"""
import sys, os, time

for _p in ('/opt/trn_rl_repo', '/root/.axon_site/_ro/trn_rl_repo'):
    if os.path.isdir(_p) and _p not in sys.path:
        sys.path.insert(0, _p)

import numpy as np

N = 500_000
E = 3_000_000
C = 128
M = 125_000
NUMPOOL = 5
NCORES = 8
MPC = M // NCORES          # 15625 clusters per core
WIN = 128
NWIN = 128                 # windows per core (balanced via snake-deal)
G = 4                      # windows per input DMA batch
NB = NWIN // G             # 32 DMA batches per core
NWP = NWIN
MP = NWP * WIN             # padded clusters per core (16384)
FA = C + 2 + 3 * NUMPOOL + 1 + 1   # 147 A-section cols
FB = 2                              # B-section: xpos_lo only
F2 = FA + FB                        # 149 input cols per node
FO = FA                             # 147 output cols


def _split_waits(nc, mybir, maxw=1):
    """This container's walrus rejects >1 sync wait per instruction; split
    extras into preceding wait-only EventSemaphore ops on the same engine."""
    for fn in nc.m.functions:
        for bb in fn.blocks:
            new_insts = []
            for inst in bb.instructions:
                si = getattr(inst, 'sync_info', None)
                if si is not None and si.on_wait and len(si.on_wait) > maxw:
                    waits = list(si.on_wait)
                    extra, keep = waits[:-maxw], waits[-maxw:]
                    for w in extra:
                        ev = mybir.InstEventSemaphore(
                            name=nc.get_next_instruction_name(),
                            engine=inst.engine, ins=[], outs=[])
                        ev.sync_info = mybir.SyncInfo(on_wait=[w], on_update=[])
                        new_insts.append(ev)
                    si.on_wait.clear()
                    for w in keep:
                        si.on_wait.append(w)
                new_insts.append(inst)
            bb.instructions[:] = new_insts


def _build_nc(T):
    import concourse.bass as bass
    import concourse.mybir as mybir
    import concourse.tile as tile
    from contextlib import ExitStack

    nc = bass.Bass()
    # batch-contiguous: row (b*128+p) holds G windows x T tiles x F2 fp16
    feats = nc.dram_tensor("feats", [NB * 128, G * T * F2], mybir.dt.float16,
                           kind="ExternalInput")
    crel = nc.dram_tensor("crel", [128, NWP * T], mybir.dt.float16,
                          kind="ExternalInput")
    iota = nc.dram_tensor("iota", [128, 128], mybir.dt.float16,
                          kind="ExternalInput")
    # row (b*128+p) holds the G windows' FO-wide raw-sum rows for slot p
    out = nc.dram_tensor("out", [NB * 128, G * FO], mybir.dt.float32,
                         kind="ExternalOutput")

    with tile.TileContext(nc) as tc:
        with ExitStack() as ctx:
            const_pool = ctx.enter_context(tc.tile_pool(name="const", bufs=1))
            in_pool = ctx.enter_context(tc.tile_pool(name="in", bufs=6))
            st_pool = ctx.enter_context(tc.tile_pool(name="st", bufs=8))
            out_pool = ctx.enter_context(tc.tile_pool(name="outp", bufs=4))
            psum_pool = ctx.enter_context(
                tc.tile_pool(name="psum", bufs=8, space="PSUM"))

            iota_t = const_pool.tile([128, 128], mybir.dt.float16)
            nc.sync.dma_start(iota_t[:], iota[:])
            crel_t = const_pool.tile([128, NWP * T], mybir.dt.float16)
            nc.sync.dma_start(crel_t[:], crel[:])

            for b in range(NB):
                ft = in_pool.tile([128, G * T * F2], mybir.dt.float16, tag="ft")
                nc.sync.dma_start(ft[:], feats[b * 128:(b + 1) * 128, :])
                og = out_pool.tile([128, G * FO], mybir.dt.float32, tag="og")
                for wi in range(G):
                    w = b * G + wi
                    fw = ft[:, wi * T * F2:(wi + 1) * T * F2]

                    # S.T for the window's T tiles in one DVE op:
                    # st[p, t, q] = (crel[p, w*T+t] == q)
                    st = st_pool.tile([128, T * 128], mybir.dt.float16,
                                      tag="st")
                    st3 = st[:].rearrange("p (t q) -> p t q", q=128)
                    crel_b = crel_t[:, w * T:(w + 1) * T].to_broadcast(
                        [128, T, 128])
                    iota_b = bass.AP(iota_t[:].tensor, iota_t[:].offset,
                                     [iota_t[:].ap[0], [0, T],
                                      iota_t[:].ap[1]])
                    nc.vector.tensor_tensor(out=st3, in0=crel_b, in1=iota_b,
                                            op=mybir.AluOpType.is_equal)

                    ps = psum_pool.tile([128, FA], mybir.dt.float32)
                    for t in range(T):
                        lhsT = st[:, t * 128:(t + 1) * 128]
                        nc.tensor.matmul(
                            out=ps[:, 0:FA], lhsT=lhsT,
                            rhs=fw[:, t * F2:t * F2 + FA],
                            start=(t == 0), stop=False)
                        # xpos_lo folds into the xpos_hi psum columns
                        nc.tensor.matmul(
                            out=ps[:, C:C + 2], lhsT=lhsT,
                            rhs=fw[:, t * F2 + FA:(t + 1) * F2],
                            start=False, stop=(t == T - 1))

                    # raw sums out; all division happens on the host
                    nc.scalar.copy(og[:, wi * FO:(wi + 1) * FO], ps[:, 0:FA])
                nc.scalar.dma_start(out[b * 128:(b + 1) * 128, :], og[:])

    import concourse.mybir as mybir2
    _split_waits(nc, mybir2)
    return nc


def _host_prepare(x, xpos, indexattten, batch, poolindex):
    poolindex = int(poolindex)
    cluster = np.asarray(indexattten)[:, poolindex].astype(np.int32)
    cl64 = cluster.astype(np.int64)
    core_of = cl64 // MPC
    c_local = cl64 - core_of * MPC

    # balance node counts across NWIN windows per core: snake-deal clusters
    # (sorted by count desc) into windows; slot-in-window = deal round.
    cnt_cc = np.zeros((NCORES, MPC), np.int32)
    np.add.at(cnt_cc, (core_of, c_local), 1)
    order_c = np.argsort(-cnt_cc, axis=1, kind='stable')
    ridx = np.arange(MPC)
    rnd = ridx // NWIN
    pos = ridx % NWIN
    w_sorted = np.where(rnd % 2 == 0, pos, NWIN - 1 - pos)
    w_c = np.empty((NCORES, MPC), np.int32)
    j_c = np.empty((NCORES, MPC), np.int32)
    rows = np.arange(NCORES)[:, None]
    w_c[rows, order_c] = w_sorted[None, :]
    j_c[rows, order_c] = rnd[None, :]

    w_local = w_c[core_of, c_local].astype(np.int64)
    c_rel = j_c[core_of, c_local].astype(np.float16)
    gw = (core_of * NWIN + w_local).astype(np.int64)

    counts = np.bincount(gw, minlength=NCORES * NWIN)
    Kmax = int(counts.max())
    K = max(128, ((Kmax + 127) // 128) * 128)
    T = K // 128
    S = NWP * K

    order = np.argsort(gw, kind='stable')
    gws = gw[order]
    starts = np.zeros(NCORES * NWIN + 1, dtype=np.int64)
    np.cumsum(counts, out=starts[1:])
    rank = np.arange(N, dtype=np.int64) - starts[gws]
    p_of = rank % 128
    t_of = rank // 128
    wl = w_local[order]
    b_of = wl // G
    wi_of = wl - b_of * G
    slot_in_core = b_of * (128 * G * T) + p_of * (G * T) + wi_of * T + t_of
    global_slot = core_of[order] * S + slot_in_core

    F16 = np.float16
    xo = np.asarray(x)[order].astype(F16)
    xpo = np.asarray(xpos)[order]
    xp_hi = xpo.astype(F16)
    xp_lo = (xpo - xp_hi.astype(np.float32)).astype(F16)
    vi = np.asarray(indexattten)[order].astype(np.int32)
    idx_parts = np.empty((N, 3 * NUMPOOL), F16)
    idx_parts[:, 0::3] = (vi >> 16).astype(F16)
    idx_parts[:, 1::3] = ((vi >> 8) & 255).astype(F16)
    idx_parts[:, 2::3] = (vi & 255).astype(F16)

    feats_pad = np.zeros((NCORES * S, F2), F16)
    feats_pad[global_slot, 0:C] = xo
    feats_pad[global_slot, C:C + 2] = xp_hi
    feats_pad[global_slot, C + 2:C + 2 + 3 * NUMPOOL] = idx_parts
    feats_pad[global_slot, FA - 2] = np.asarray(batch)[order].astype(F16)
    feats_pad[global_slot, FA - 1] = F16(1.0)
    feats_pad[global_slot, FA:FA + 2] = xp_lo
    feats_pad = feats_pad.reshape(NCORES, NB * 128, G * T * F2)

    crel_slot = np.full(NCORES * S, -1.0, np.float16)
    crel_slot[global_slot] = c_rel[order]
    crel_T = np.ascontiguousarray(
        crel_slot.reshape(NCORES, NB, 128, G, T).transpose(0, 2, 1, 3, 4)
        .reshape(NCORES, 128, NWP * T))

    iota = np.tile(np.arange(128, dtype=np.float16), (128, 1))
    # device flat row (out viewed as [(NB*128*G), FO]) of local cluster c:
    # row = (b*128 + j)*G + wi with b = w//G, wi = w%G
    w64 = w_c.astype(np.int64)
    out_row = ((w64 // G) * 128 + j_c) * G + (w64 % G)
    return cluster, feats_pad, crel_T, iota, T, out_row


_nc_cache = {}


def _run_device(feats_pad, crel_T, iota, T, trace=False, tmpdir=None):
    from concourse.bass_utils import run_bass_kernel_spmd
    if T not in _nc_cache:
        _nc_cache[T] = _build_nc(T)
    nc = _nc_cache[T]
    in_maps = [
        {"feats": feats_pad[i], "crel": crel_T[i], "iota": iota}
        for i in range(NCORES)
    ]
    return run_bass_kernel_spmd(nc, in_maps, core_ids=list(range(NCORES)),
                                trace=trace, tmpdir=tmpdir)


def _segment_sums_numpy(x, xpos, indexattten, batch, cluster):
    """Host fallback producing the same [M, FO] raw-sum layout."""
    big = np.zeros((M, FO), np.float32)
    feats = np.concatenate([
        np.asarray(x, np.float32),
        np.asarray(xpos, np.float32),
        np.zeros((N, 3 * NUMPOOL), np.float32),
        np.asarray(batch, np.float32)[:, None],
        np.ones((N, 1), np.float32)], axis=1)
    vi = np.asarray(indexattten).astype(np.int32)
    feats[:, C + 2:C + 2 + 3 * NUMPOOL:3] = (vi >> 16).astype(np.float32)
    feats[:, C + 3:C + 2 + 3 * NUMPOOL:3] = ((vi >> 8) & 255).astype(np.float32)
    feats[:, C + 4:C + 2 + 3 * NUMPOOL:3] = (vi & 255).astype(np.float32)
    np.add.at(big, cluster, feats)
    return big


def kernel(x, edge_index, xpos, indexattten, batch, poolindex,
           trace=False, tmpdir=None, _times=None):
    x = np.asarray(x)
    edge_index = np.asarray(edge_index)
    xpos = np.asarray(xpos)
    indexattten = np.asarray(indexattten)
    batch = np.asarray(batch)

    t0 = time.time()
    cluster, feats_pad, crel_T, iota, T, out_row = _host_prepare(
        x, xpos, indexattten, batch, poolindex)
    t1 = time.time()

    big = None
    res = None
    try:
        res = _run_device(feats_pad, crel_T, iota, T, trace=trace,
                          tmpdir=tmpdir)
        big = np.concatenate(
            [res.results[i]["out"].reshape(NB * 128 * G, FO)[out_row[i]]
             for i in range(NCORES)], axis=0)
    except Exception as e:
        sys.stderr.write(f"[kernel] device path failed ({e!r}); "
                         f"falling back to host compute\n")
        big = _segment_sums_numpy(x, xpos, indexattten, batch, cluster)
    t2 = time.time()

    counts = big[:, FO - 1:FO]
    denom = np.maximum(counts, np.float32(1.0))
    ip = big[:, C + 2:C + 2 + 3 * NUMPOOL]
    idx_sums = (ip[:, 0::3] * np.float32(65536.0)
                + ip[:, 1::3] * np.float32(256.0) + ip[:, 2::3])
    indexatttennew = (idx_sums / denom).astype(np.int64)
    new_batch = (big[:, FO - 2:FO - 1] / denom)[:, 0].astype(np.int64)
    new_xfinal = big[:, :C] / denom
    new_pos = big[:, C:C + 2] / denom
    t3 = time.time()

    cl64 = cluster.astype(np.int64)
    keys = cl64[edge_index[0]] * M + cl64[edge_index[1]]
    uk = np.unique(keys)
    full = np.empty(E, dtype=np.int64)
    full[:len(uk)] = uk
    full[len(uk):] = uk[-1]
    new_edge_index = np.stack([full // M, full % M])
    t4 = time.time()
    if _times is not None:
        _times.update(dict(prep=t1 - t0, device=t2 - t1, post=t3 - t2,
                           edges=t4 - t3))
    out = (new_xfinal, new_edge_index, new_pos, indexatttennew, new_batch,
           edge_index, cluster, batch)
    if trace:
        return out, res
    return out
